# revision 2
# baseline (speedup 1.0000x reference)
"""Trainium2 Bass kernel for nn_CapsuleLayer — channel-sharded v5.

v4 -> v5: collectives avoid the AllReduce firmware floor (~9.7us on 8
cores).  Routing iterations AllGather the bf16 s-partials (~4.6us floor)
and reduce the 8 shards locally on DVE/Pool; the final iteration uses
AllToAll (~4.7us floor) + a selection-matmul partition reduce, and each
core squashes + outputs only its own 32-batch slice.

Sharding: C split 8 ways (144 channels / core), K = 1152 -> 9 chunks of 128
with partition p = (c%16)*8 + i.  Per iteration the s_j partial is
reduced across cores in bf16; squash is replicated (iters 0-1).  Routing
logits are channel-local, held replicated over the 8 i-partitions.
"""

import numpy as np

import concourse.bass as bass
import concourse.bacc as bacc
import concourse.tile as tile
from concourse import mybir
from concourse import bass_utils

# ------------------------------------------- custom DVE op: prefix(W*G)


def _register_mulscan():
    import numpy as np
    from concourse import dve_ops
    from concourse.dve_spec import Spec, Src0, Src1, AluOp, scan, lower
    from concourse.dve_uop import DveOpSpec

    name = "CAPS_MULSCAN_ANT"
    for op in dve_ops.OPS:
        if op.name == name:
            return op
    spec = Spec(
        body=scan(AluOp.ADD, Src0 * Src1),
        reference=lambda in0, in1, s0, s1, imm2: np.cumsum(
            np.asarray(in0, np.float32).reshape(in0.shape[0], -1)
            * np.asarray(in1, np.float32).reshape(in1.shape[0], -1),
            axis=1,
        ),
    )
    row = dve_ops._CUSTOM_DVE_ROW_BASE + len(dve_ops.OPS)
    shas = {}
    for ver in ("v3", "v4"):
        s = DveOpSpec(name=name, opcode=row, uops=lower(spec, ver=ver), rd1_en=True)
        shas[ver] = s.sha(ver)
    op = dve_ops.DveOp(name, spec, subdim=False, uops_sha=shas)
    dve_ops.OPS.append(op)
    dve_ops.CUSTOM_DVE_SPECS[name] = spec
    dve_ops._SUB_OPCODE_FOR_NAME[name] = row
    return op


MULSCAN = _register_mulscan()


def _pin_act_tables():
    """Make natural_log_exp_and_others the unique candidate set for
    exp/ln/square so bacc's table-load pass never alternates sets."""
    import functools
    import concourse.bacc as _bacc
    import concourse.hw_specs as _hw
    if getattr(_bacc, "_caps_act_pinned", False):
        return
    orig = _hw.get_activation_tables

    @functools.cache
    def pinned(module_arch):
        tables = dict(orig(module_arch))
        keep = "natural_log_exp_and_others"
        assert keep in tables
        only = tables[keep]
        excl = {f for f in only}
        out = {}
        for name, funcs in tables.items():
            if name == keep:
                out[name] = funcs
            else:
                out[name] = funcs - excl
        return out

    _bacc.get_activation_tables = pinned
    _hw.get_activation_tables_orig = orig
    _bacc._caps_act_pinned = True


_pin_act_tables()

# ---------------------------------------------------------------- constants
B, I, C, U, S = 256, 8, 1152, 10, 16
NCORES = 8
CL = C // NCORES            # 144 channels per core
KT = CL * I // 128          # 9 K-chunks of 128 (16 c x 8 i)
NUS = U * S                 # 160
NSEG = KT * U               # 90 (chunk, u) segments
EPS = 1e-8
NUM_ROUTING = 3

MM_CFG = "bf16"

_DT = {
    "f32": mybir.dt.float32,
    "bf16": mybir.dt.bfloat16,
}


def _np_dt(cfg):
    if cfg == "bf16":
        import ml_dtypes
        return ml_dtypes.bfloat16
    return np.float32


# ---------------------------------------------------------------- device code
def build_nc(cfg=MM_CFG, repeat=1, collectives=True, final_rs=True, solo=False):
    nc = bacc.Bacc(
        "TRN2",
        target_bir_lowering=False,
        debug=False,
        num_devices=1 if collectives is False else NCORES,
    )
    mdt = _DT[cfg]
    f32 = mybir.dt.float32

    w_d = nc.dram_tensor("w_sb", [128, KT * NUS], mdt, kind="ExternalInput")
    xt_d = nc.dram_tensor("x_t", [128, KT * B], mdt, kind="ExternalInput")
    xb_d = nc.dram_tensor("x_b", [128, 2 * KT * 128], mdt, kind="ExternalInput")
    selx_d = nc.dram_tensor("selx", [128, 128], f32, kind="ExternalInput")
    sel16_d = nc.dram_tensor("sel16", [128, 16], mdt, kind="ExternalInput")
    out_d = nc.dram_tensor("v_out", [16, 2 * NUS], f32, kind="ExternalOutput")

    with tile.TileContext(nc) as tc:
        with (
            tc.tile_pool(name="singles", bufs=1) as singles,
            tc.tile_pool(name="work", bufs=2) as work,
            tc.tile_pool(name="small", bufs=2) as small,
            tc.tile_pool(name="ps_s", bufs=1, space="PSUM") as ps_s,
            tc.tile_pool(name="ps_g", bufs=1, space="PSUM") as ps_g,
            tc.tile_pool(name="ps_b", bufs=1, space="PSUM") as ps_b,
            tc.tile_pool(name="dram", bufs=2, space="DRAM") as dram,
        ):
            # ---------------- persistent SBUF loads; x_t/w first (s-pass),
            # x_b (first G use) and selx (first b-update) later
            x_t = singles.tile([128, KT, B], mdt)
            w_sb = singles.tile([128, KT, U, S], mdt)
            for r in range(3):
                nc.sync.dma_start(
                    out=x_t[:, r * 3:(r + 1) * 3, :],
                    in_=xt_d[:, r * 3 * B:(r + 1) * 3 * B],
                )
                nc.sync.dma_start(
                    out=w_sb[:, r * 3:(r + 1) * 3, :, :],
                    in_=w_d[:, r * 3 * NUS:(r + 1) * 3 * NUS],
                )
            x_b = singles.tile([128, 2, KT, 128], mdt)
            nc.sync.dma_start(out=x_b[:], in_=xb_d[:])
            selx = singles.tile([128, 128], f32)
            nc.sync.dma_start(out=selx[:], in_=selx_d[:])
            sel16 = singles.tile([128, 16], mdt)
            nc.sync.dma_start(out=sel16[:], in_=sel16_d[:])
            eps_sb = singles.tile([128, 1], f32)
            nc.vector.memset(eps_sb[:], EPS)

            def squash(s_ap, alpha, vdt, P, H, tg):
                """v = squash(alpha * s);  s_ap [P, H, U, S]."""
                s2 = small.tile([P, H, U, S], f32, tag=f"s2{tg}")
                nc.scalar.activation(
                    out=s2[:], in_=s_ap,
                    func=mybir.ActivationFunctionType.Square,
                )
                sq = small.tile([P, H, U], f32, tag=f"sq{tg}")
                nc.vector.reduce_sum(out=sq[:], in_=s2[:], axis=mybir.AxisListType.X)
                if alpha != 1.0:
                    t = small.tile([P, H, U], f32, tag=f"t{tg}")
                    nc.vector.tensor_scalar_mul(t[:], sq[:], alpha * alpha)
                else:
                    t = sq
                lnt = small.tile([P, H, U], f32, tag=f"lnt{tg}")
                nc.scalar.activation(
                    out=lnt[:], in_=t[:],
                    func=mybir.ActivationFunctionType.Ln, bias=eps_sb[:P, :],
                )
                rt = small.tile([P, H, U], f32, tag=f"rt{tg}")
                nc.scalar.activation(
                    out=rt[:], in_=lnt[:],
                    func=mybir.ActivationFunctionType.Exp, scale=0.5,
                )
                dd = small.tile([P, H, U], f32, tag=f"dd{tg}")
                nc.vector.scalar_tensor_tensor(
                    out=dd[:], in0=t[:], scalar=1.0, in1=rt[:],
                    op0=mybir.AluOpType.add, op1=mybir.AluOpType.mult,
                )
                g = small.tile([P, H, U], f32, tag=f"g{tg}")
                nc.vector.reciprocal(g[:], dd[:])
                af = small.tile([P, H, U], f32, tag=f"af{tg}")
                nc.vector.scalar_tensor_tensor(
                    out=af[:], in0=t[:], scalar=float(alpha), in1=g[:],
                    op0=mybir.AluOpType.mult, op1=mybir.AluOpType.mult,
                )
                v = small.tile([P, H, U, S], vdt, tag=f"v{tg}")
                nc.vector.tensor_mul(
                    v[:], s_ap,
                    af[:, :, :, None].broadcast_to([P, H, U, S]),
                )
                return v

            # ------------------------------------------------ routing loop
            for _rep in range(repeat):
                c_sm = None         # [128, KT, U] f32 softmax'd coupling
                b_sb = None         # [128, NSEG] f32 logits (i-replicated)
                for it in range(NUM_ROUTING):
                    alpha = 1.0 / U if it == 0 else 1.0
                    # ---------------- weff = W * c  (skip on it 0: c uniform)
                    if it == 0:
                        weff = w_sb
                    else:
                        weff = work.tile([128, KT, U, S], mdt, tag="weff")
                        hk = KT // 2 + 1  # 5 chunks on vector, 4 on gpsimd
                        nc.vector.tensor_mul(
                            weff[:, :hk],
                            w_sb[:, :hk],
                            c_sm[:, :hk, :, None].broadcast_to([128, hk, U, S]),
                        )
                        nc.gpsimd.tensor_mul(
                            weff[:, hk:],
                            w_sb[:, hk:],
                            c_sm[:, hk:, :, None].broadcast_to(
                                [128, KT - hk, U, S]
                            ),
                        )
                    weff_flat = weff[:].rearrange("p k u s -> p (k u s)")
                    last = it == NUM_ROUTING - 1
                    # ---------------- s partial: out[b, us] over 2 b-halves
                    s_ps = ps_s.tile([128, 2, 512], f32, tag="s")
                    for h in range(2):
                        for kb in range(KT):
                            nc.tensor.matmul(
                                out=s_ps[:, h, :NUS],
                                lhsT=x_t[:, kb, h * 128:(h + 1) * 128],
                                rhs=weff_flat[:, kb * NUS:(kb + 1) * NUS],
                                start=(kb == 0),
                                stop=(kb == KT - 1),
                            )
                    if last:
                        # ---------------- final iteration: AllToAll; each
                        # core reduces + squashes only its own 32 batches
                        s_stage = work.tile([128, 2, NUS], mdt, tag="s_stagef")
                        nc.scalar.copy(out=s_stage[:], in_=s_ps[:, :, :NUS])
                        ar_in = dram.tile([128, 2 * NUS], mdt, tag="ar_inf")
                        nc.sync.dma_start(
                            out=ar_in[:],
                            in_=s_stage[:].rearrange("p h m -> p (h m)"),
                        )
                        ar_out = dram.tile([128, 2 * NUS], mdt, tag="ar_outf")
                        if collectives is True:
                            nc.gpsimd.collective_compute(
                                "AllToAll",
                                mybir.AluOpType.bypass,
                                replica_groups=[[i] for i in range(NCORES)] if solo else [list(range(NCORES))],
                                ins=[ar_in[:].opt()],
                                outs=[ar_out[:].opt()],
                            )
                        else:
                            nc.sync.dma_start(out=ar_out[:], in_=ar_in[:])
                        a2a_sb = work.tile([128, 2 * NUS], mdt, tag="a2a_sb")
                        nc.sync.dma_start(out=a2a_sb[:], in_=ar_out[:])
                        s16_ps = ps_b.tile([16, 2 * NUS], f32, tag="s16")
                        nc.tensor.matmul(
                            out=s16_ps[:], lhsT=sel16[:], rhs=a2a_sb[:],
                            start=True, stop=True,
                        )
                        v = squash(
                            s16_ps[:].rearrange("p (h u s) -> p h u s", h=2, u=U),
                            alpha, f32, 16, 2, "f",
                        )
                        nc.sync.dma_start(
                            out=out_d[:],
                            in_=v[:].rearrange("p h u s -> p (h u s)"),
                        )
                        break
                    # ---------------- single AllReduce (bf16), batched squash
                    s_stage = work.tile([128, 2, NUS], mdt, tag="s_stage")
                    nc.scalar.copy(out=s_stage[:], in_=s_ps[:, :, :NUS])
                    ar_in = dram.tile([128, 2 * NUS], mdt, tag=f"ar_in{it}")
                    nc.sync.dma_start(
                        out=ar_in[:],
                        in_=s_stage[:].rearrange("p h m -> p (h m)"),
                    )
                    ar_out = dram.tile([128, 2 * NUS], mdt, tag=f"ar_out{it}", addr_space="Shared")
                    if collectives is True:
                        nc.gpsimd.collective_compute(
                            "AllReduce",
                            mybir.AluOpType.add,
                            replica_groups=[[i] for i in range(NCORES)] if solo else [list(range(NCORES))],
                            ins=[ar_in[:].opt()],
                            outs=[ar_out[:].opt()],
                        )
                    else:
                        nc.sync.dma_start(out=ar_out[:], in_=ar_in[:])
                    s_sb = work.tile([128, 2, U, S], mdt, tag="s_sb")
                    nc.sync.dma_start(out=s_sb[:], in_=ar_out[:])
                    v = squash(s_sb[:], alpha, mdt, 128, 2, "r")
                    g_ps = ps_g.tile([128, 3, 512], f32, tag="g")
                    for kb in range(KT):
                        for h in range(2):
                            nc.tensor.matmul(
                                out=g_ps[
                                    :, kb // 3,
                                    (kb % 3) * NUS:(kb % 3) * NUS + NUS,
                                ],
                                lhsT=x_b[:, h, kb, :],
                                rhs=v[:, h].rearrange("p u s -> p (u s)"),
                                start=(h == 0),
                                stop=(h == 1),
                            )
                    # fused product + prefix-sum over (u s) per chunk-group
                    pref = work.tile([128, 16 * (NSEG + 1)], f32, tag="pref")
                    nc.scalar.mul(out=pref[:, 0:1], in_=eps_sb[:, 0:1], mul=0.0)
                    nc.vector._custom_dve(
                        MULSCAN,
                        out=pref[:, 1:1 + KT * NUS],
                        in0=w_sb[:].rearrange("p k u s -> p (k u s)").rearrange(
                            "p (a m) -> p a m", a=3
                        ),
                        in1=g_ps[:, :, :3 * NUS],
                    )
                    ends = pref[:, S:S + NSEG * S].rearrange(
                        "p (n s) -> p n s", s=S
                    )[:, :, 0]
                    prevs = pref[:, 0:NSEG * S].rearrange(
                        "p (n s) -> p n s", s=S
                    )[:, :, 0]
                    d = small.tile([128, NSEG], f32, tag="d")
                    nc.vector.scalar_tensor_tensor(
                        out=d[:], in0=prevs, scalar=-1.0, in1=ends,
                        op0=mybir.AluOpType.mult, op1=mybir.AluOpType.add,
                    )
                    # group-sum over the 8 i-partitions, replicated back to
                    # all 128 partitions (selx[p,q] = (p//8==q//8)/B)
                    b_ps = ps_b.tile([128, NSEG], f32, tag="b_ps")
                    nc.tensor.matmul(
                        out=b_ps[:], lhsT=selx[:], rhs=d[:],
                        start=True, stop=True,
                    )
                    # ---------------- b update + softmax over u (local)
                    b_new = small.tile([128, NSEG], f32, tag=f"b{it}")
                    if b_sb is None:
                        nc.scalar.copy(out=b_new[:], in_=b_ps[:])
                    else:
                        nc.vector.tensor_add(b_new[:], b_sb[:], b_ps[:])
                    b_sb = b_new
                    e = small.tile([128, KT, U], f32, tag="e")
                    nc.scalar.activation(
                        out=e[:],
                        in_=b_sb[:].rearrange("p (k u) -> p k u", u=U),
                        func=mybir.ActivationFunctionType.Exp,
                    )
                    se = small.tile([128, KT], f32, tag="se")
                    nc.vector.reduce_sum(
                        out=se[:], in_=e[:], axis=mybir.AxisListType.X
                    )
                    re = small.tile([128, KT], f32, tag="re")
                    nc.vector.reciprocal(re[:], se[:])
                    c_sm = small.tile([128, KT, U], f32, tag="c_sm")
                    nc.vector.tensor_mul(
                        c_sm[:], e[:], re[:, :, None].broadcast_to([128, KT, U])
                    )

    nc.compile()
    return nc


# ---------------------------------------------------------------- host prep
def prep_inputs(x, weight, cfg=MM_CFG):
    """Full inputs -> per-core in_maps with kernel-ready layouts."""
    x = np.asarray(x, dtype=np.float32)
    weight = np.asarray(weight, dtype=np.float32)
    npdt = _np_dt(cfg)

    selx = np.zeros((128, 128), np.float32)
    pp = np.arange(128)
    selx[:, :] = (pp[:, None] // 8 == pp[None, :] // 8) / B
    sel16 = (pp[:, None] % 16 == np.arange(16)[None, :]).astype(npdt)

    in_maps = []
    for k in range(NCORES):
        cs = slice(k * CL, (k + 1) * CL)
        w = (
            weight[cs]
            .reshape(KT, 16, U, S, I)
            .transpose(1, 4, 0, 2, 3)          # [16, I, KT, U, S]
            .reshape(128, KT * U * S)
        )
        xs = x[:, :, cs]                        # [B, I, CL]
        x_t = (
            xs.transpose(2, 1, 0)               # [CL, I, B]
            .reshape(KT, 16, I, B)
            .transpose(1, 2, 0, 3)              # [16, I, KT, B]
            .reshape(128, KT * B)
        )
        x_b = (
            xs.transpose(0, 2, 1)               # [B, CL, I]
            .reshape(2, 128, KT, 16 * I)
            .transpose(1, 0, 2, 3)
            .reshape(128, 2 * KT * 128)
        )
        in_maps.append({
            "w_sb": np.ascontiguousarray(w, dtype=npdt),
            "x_t": np.ascontiguousarray(x_t, dtype=npdt),
            "x_b": np.ascontiguousarray(x_b, dtype=npdt),
            "selx": selx,
            "sel16": sel16,
        })
    return in_maps


def assemble_output(results):
    # ReduceScatter leaves rank r with batches {h*128 + 16r + p}
    out = np.empty((B, U, S, 1), np.float32)
    for r in range(NCORES):
        v = results[r]["v_out"].astype(np.float32).reshape(16, 2, U, S)
        for h in range(2):
            out[h * 128 + 16 * r:h * 128 + 16 * r + 16] = v[:, h][..., None]
    return out


_NC_CACHE = {}


def _get_nc(cfg=MM_CFG):
    if cfg not in _NC_CACHE:
        _NC_CACHE[cfg] = build_nc(cfg)
    return _NC_CACHE[cfg]


def kernel(x, weight):
    nc = _get_nc()
    in_maps = prep_inputs(x, weight)
    res = bass_utils.run_bass_kernel_spmd(
        nc, in_maps, core_ids=list(range(NCORES))
    )
    return assemble_output(res.results)
